# revision 14
# baseline (speedup 1.0000x reference)
"""Trainium2 Bass kernel for AlignedMPNN (gnn_message_passing).

Data-parallel over batch B=8 across 8 NeuronCores; each core computes one
batch element end-to-end (no collectives). Per core:

  et  = concat([edge_fts, e_hidden], -1) @ We + be          # [N*N, D]  (heavy)
  nt  = concat([node_fts, hidden], -1) (+ virtual zero row) # [N+1, 2F]
  msg1 = nt @ W_m1 + b_m1 ; msg2 = nt @ W_m2 + b_m2
  max2[j, d] = max_i (msg2[i, d] + (adjT[j, i] - 1) * 1e6)  # masked max
  ret = nt @ W_o1 + b_o1 + (msg1 + max2) @ W_o2 + b_o2      # rows 0..N-1

The et matmul contracts over the feature dim, so edge tiles are cast to
bf16 and transposed on the TensorEngine (1 cyc/row + fast weight load),
then multiplied by bf16 We with fp32 PSUM accumulation. The bias is
folded into the PSUM->SBUF drain as a tensor_tensor add against a
broadcast bias tile. Edge rows map to partitions contiguously
(row = chunk*CH + p*RT + r) so every DMA descriptor covers RT*512 bytes;
the same permutation is used on load and store so DRAM layout is exact.

Masked max runs on the VectorEngine as one fused scalar_tensor_tensor
per (i, j-tile); msg2 rows are broadcast across partitions by K=1
matmuls against a ones row (msg2 is flattened onto partition 0 via a
DRAM bounce so rows are base-partition-0 slices).
"""

import numpy as np
from contextlib import ExitStack

import concourse.bass as bass
from concourse import bacc
import concourse.tile as tile
from concourse import mybir
from concourse.masks import make_identity

B, N, F, D = 8, 256, 128, 128
P = 128
TF = 2 * F          # 256 = concat feature dim
NROW = N * N        # 65536 edge rows per batch element
CH = 1024           # edge rows per pipeline chunk
RT = CH // P        # 8 row-tiles per chunk
HT = RT // 2        # 4 row-tiles per half chunk (one PSUM bank)
NCH = NROW // CH    # 64 chunks
NODES = N + 1       # 257 (with virtual node)
BIGC = 1000000.0
NEG = -1.0e30
f32 = mybir.dt.float32
bf16 = mybir.dt.bfloat16
i32 = mybir.dt.int32
ADD = mybir.AluOpType.add
MAX = mybir.AluOpType.max
MULT = mybir.AluOpType.mult


def build(nch=NCH):
    nc = bacc.Bacc(None)

    node = nc.declare_dram_parameter("node_fts", [N, F], f32, isOutput=False)
    hid = nc.declare_dram_parameter("hidden", [N, F], f32, isOutput=False)
    edge = nc.declare_dram_parameter("edge_fts", [NROW, F], f32, isOutput=False)
    ehid = nc.declare_dram_parameter("e_hidden", [NROW, F], f32, isOutput=False)
    adj = nc.declare_dram_parameter("adj_mat", [N, N], i32, isOutput=False)
    We = nc.declare_dram_parameter("We", [TF, D], f32, isOutput=False)
    be = nc.declare_dram_parameter("be", [1, D], f32, isOutput=False)
    Wm1 = nc.declare_dram_parameter("W_m1", [TF, D], f32, isOutput=False)
    bm1 = nc.declare_dram_parameter("b_m1", [1, D], f32, isOutput=False)
    Wm2 = nc.declare_dram_parameter("W_m2", [TF, D], f32, isOutput=False)
    bm2 = nc.declare_dram_parameter("b_m2", [1, D], f32, isOutput=False)
    Wo1 = nc.declare_dram_parameter("W_o1", [TF, D], f32, isOutput=False)
    bo1 = nc.declare_dram_parameter("b_o1", [1, D], f32, isOutput=False)
    Wo2 = nc.declare_dram_parameter("W_o2", [D, D], f32, isOutput=False)
    bo2 = nc.declare_dram_parameter("b_o2", [1, D], f32, isOutput=False)
    out_et = nc.declare_dram_parameter("out_et", [NROW, D], f32, isOutput=True)
    out_ret = nc.declare_dram_parameter("out_ret", [N, D], f32, isOutput=True)
    msg2_dram = nc.dram_tensor("msg2_scratch", [N, D], bf16)

    with ExitStack() as ctx:
        tc = ctx.enter_context(tile.TileContext(nc))
        const = ctx.enter_context(tc.tile_pool(name="const", bufs=1))
        small = ctx.enter_context(tc.tile_pool(name="small", bufs=1))
        bfp = ctx.enter_context(tc.tile_pool(name="bfp", bufs=3))
        ctp = ctx.enter_context(tc.tile_pool(name="ctp", bufs=3))
        outp = ctx.enter_context(tc.tile_pool(name="outp", bufs=4))
        ps_t = ctx.enter_context(tc.tile_pool(name="ps_t", bufs=3, space="PSUM"))
        ps_mm = ctx.enter_context(tc.tile_pool(name="ps_mm", bufs=2, space="PSUM"))
        ps_sm = ctx.enter_context(tc.tile_pool(name="ps_sm", bufs=1, space="PSUM"))
        ps_bc = ctx.enter_context(tc.tile_pool(name="ps_bc", bufs=2, space="PSUM"))

        # ---- constants / weights ----
        ident = const.tile([P, P], f32)
        make_identity(nc, ident)
        ident_bf = const.tile([P, P], bf16)
        make_identity(nc, ident_bf)
        ones = const.tile([1, NODES], f32)
        nc.gpsimd.memset(ones, 1.0)
        ones_bf = const.tile([1, NODES], bf16)
        nc.gpsimd.memset(ones_bf, 1.0)

        def load_w2(dram_ap, name):
            t = const.tile([P, 2, D], f32, tag=name)
            nc.sync.dma_start(t, dram_ap.rearrange("(k p) d -> p k d", p=P))
            return t

        We_sb = load_w2(We, "We_sb")
        We_bf = const.tile([P, 2, D], bf16)
        nc.vector.tensor_copy(We_bf, We_sb)
        Wm1_sb = load_w2(Wm1, "Wm1_sb")
        Wm2_sb = load_w2(Wm2, "Wm2_sb")
        Wo1_sb = load_w2(Wo1, "Wo1_sb")
        Wo2_sb = const.tile([P, D], f32)
        nc.sync.dma_start(Wo2_sb, Wo2[:, :])

        def load_b(dram_ap, name):
            t = const.tile([1, D], f32, tag=name)
            nc.sync.dma_start(t, dram_ap[:, :])
            return t

        be_sb = load_b(be, "be_sb")
        bm1_sb = load_b(bm1, "bm1_sb")
        bm2_sb = load_b(bm2, "bm2_sb")
        bo1_sb = load_b(bo1, "bo1_sb")
        bo2_sb = load_b(bo2, "bo2_sb")

        be_bf = const.tile([1, D], bf16)
        nc.vector.tensor_copy(be_bf, be_sb)

        # ---- small inputs ----
        nf_sb = small.tile([P, 2, F], f32)
        nc.sync.dma_start(nf_sb, node.rearrange("(t p) f -> p t f", p=P))
        hd_sb = small.tile([P, 2, F], f32)
        nc.sync.dma_start(hd_sb, hid.rearrange("(t p) f -> p t f", p=P))
        adj_i = small.tile([P, 2, N], i32)
        nc.sync.dma_start(adj_i, adj.rearrange("(t p) j -> p t j", p=P))
        adjf = small.tile([P, 2, N], f32)
        nc.vector.tensor_copy(adjf[:, 0, :], adj_i[:, 0, :])
        nc.vector.tensor_copy(adjf[:, 1, :], adj_i[:, 1, :])

        # ---- ntT[f, j]: transposed node features (f split in 2 halves) ----
        ntT = small.tile([P, 2, NODES], f32)
        nc.gpsimd.memset(ntT[:, 0, N : N + 1], 0.0)
        nc.gpsimd.memset(ntT[:, 1, N : N + 1], 0.0)
        for half, src in ((0, nf_sb), (1, hd_sb)):
            for jt in range(2):
                ps = ps_sm.tile([P, P], f32, tag="sm")
                nc.tensor.transpose(ps, src[:, jt, :], ident)
                nc.scalar.copy(ntT[:, half, jt * P : (jt + 1) * P], ps)

        # ---- Abias[j, i] = (adjT_ext[j, i] - 1) * BIGC  (i=256 col -> 0) ----
        Abias = small.tile([P, 2, NODES], f32)
        nc.gpsimd.memset(Abias[:, 0, N : N + 1], 0.0)
        nc.gpsimd.memset(Abias[:, 1, N : N + 1], 0.0)
        for it in range(2):
            for jt in range(2):
                ps = ps_sm.tile([P, P], f32, tag="sm")
                nc.tensor.transpose(ps, adjf[:, it, jt * P : (jt + 1) * P], ident)
                nc.vector.tensor_scalar(
                    Abias[:, jt, it * P : (it + 1) * P], ps, BIGC, -BIGC, MULT, ADD
                )

        # ---- msg1 (j-tiles 0,1) and msg2 ----
        msg1_sb = small.tile([P, 2, D], f32)
        msg2_sb = small.tile([P, 2, D], f32)
        for W_sb, b_sb, dst in ((Wm1_sb, bm1_sb, msg1_sb), (Wm2_sb, bm2_sb, msg2_sb)):
            for jt in range(2):
                ps = ps_sm.tile([P, D], f32, tag="sm")
                jsl = ntT[:, :, jt * P : (jt + 1) * P]
                nc.tensor.matmul(ps, jsl[:, 0, :], W_sb[:, 0, :], start=True, stop=False)
                nc.tensor.matmul(ps, jsl[:, 1, :], W_sb[:, 1, :], start=False, stop=False)
                nc.tensor.matmul(ps, ones[:, :P], b_sb, start=False, stop=True)
                nc.scalar.copy(dst[:, jt, :], ps)

        # ---- flatten msg2 (bf16) onto partition 0 via DRAM bounce ----
        msg2_bf = small.tile([P, 2, D], bf16)
        nc.vector.tensor_copy(msg2_bf, msg2_sb)
        msg2_flat = small.tile([1, NODES * D], bf16)
        nc.sync.dma_start(
            msg2_dram[:, :].rearrange("(t p) d -> p t d", p=P), msg2_bf
        )
        nc.sync.dma_start(
            msg2_flat[0:1, : N * D], msg2_dram[:, :].rearrange("r d -> (r d)")
        )
        bm2_bf = small.tile([1, D], bf16)
        nc.vector.tensor_copy(bm2_bf, bm2_sb)
        nc.scalar.copy(msg2_flat[0:1, N * D : NODES * D], bm2_bf)

        # ---- masked max over i: acc[j, d] ----
        acc = small.tile([P, 2, D], f32)
        nc.gpsimd.memset(acc, NEG)
        for i in range(NODES):
            bc = ps_bc.tile([P, D], f32, tag="bc")
            nc.tensor.matmul(
                bc, ones_bf[:, :P], msg2_flat[0:1, i * D : (i + 1) * D],
                start=True, stop=True,
            )
            for jt in range(2):
                nc.vector.scalar_tensor_tensor(
                    acc[:, jt, :], bc, Abias[:, jt, i : i + 1], acc[:, jt, :], ADD, MAX
                )

        # ---- msgs = msg1 + acc; ret = nt@Wo1 + msgs@Wo2 + bo1 + bo2 ----
        msgs = small.tile([P, 2, D], f32)
        msgsT = small.tile([P, 2, P], f32)
        for jt in range(2):
            nc.vector.tensor_add(msgs[:, jt, :], msg1_sb[:, jt, :], acc[:, jt, :])
            ps = ps_sm.tile([P, P], f32, tag="sm")
            nc.tensor.transpose(ps, msgs[:, jt, :], ident)
            nc.scalar.copy(msgsT[:, jt, :], ps)
        for jt in range(2):
            ps = ps_sm.tile([P, D], f32, tag="sm")
            jsl = ntT[:, :, jt * P : (jt + 1) * P]
            nc.tensor.matmul(ps, jsl[:, 0, :], Wo1_sb[:, 0, :], start=True, stop=False)
            nc.tensor.matmul(ps, jsl[:, 1, :], Wo1_sb[:, 1, :], start=False, stop=False)
            nc.tensor.matmul(ps, msgsT[:, jt, :], Wo2_sb, start=False, stop=False)
            nc.tensor.matmul(ps, ones[:, :P], bo1_sb, start=False, stop=False)
            nc.tensor.matmul(ps, ones[:, :P], bo2_sb, start=False, stop=True)
            rsb = small.tile([P, D], f32, tag="rsb")
            nc.scalar.copy(rsb, ps)
            nc.sync.dma_start(out_ret[jt * P : (jt + 1) * P, :], rsb)

        # ---- et stream: casting DMA (f32->bf16) -> PE transpose -> matmul
        #      (+ K=1 bias matmul) -> drain -> store.
        #      Row permutation: row = c*CH + p*RT + r
        for c in range(nch):
            rows = slice(c * CH, (c + 1) * CH)
            ebf = bfp.tile([P, RT, F], bf16, tag="ebf")
            nc.gpsimd.dma_start(ebf, edge[rows, :].rearrange("(p r) f -> p r f", r=RT))
            hbf = bfp.tile([P, RT, F], bf16, tag="hbf")
            nc.gpsimd.dma_start(hbf, ehid[rows, :].rearrange("(p r) f -> p r f", r=RT))

            cte = ctp.tile([P, RT, P], bf16, tag="cte")
            cth = ctp.tile([P, RT, P], bf16, tag="cth")
            for src, dst, eng in ((ebf, cte, 0), (hbf, cth, 1)):
                for h in range(2):
                    ps = ps_t.tile([P, HT, P], bf16, tag="pst")
                    for r in range(HT):
                        nc.tensor.transpose(ps[:, r, :], src[:, h * HT + r, :], ident_bf)
                    sl = dst[:, h * HT : (h + 1) * HT, :]
                    if eng == 0 and h == 0:
                        nc.vector.tensor_copy(sl, ps)
                    else:
                        nc.scalar.copy(sl, ps)

            osb = outp.tile([P, RT, D], f32, tag="osb")
            for h in range(2):
                om = ps_mm.tile([P, HT, D], f32, tag="om")
                for r in range(HT):
                    rr = h * HT + r
                    nc.tensor.matmul(
                        om[:, r, :], cte[:, rr, :], We_bf[:, 0, :],
                        start=True, stop=False,
                    )
                    nc.tensor.matmul(
                        om[:, r, :], cth[:, rr, :], We_bf[:, 1, :],
                        start=False, stop=False,
                    )
                    nc.tensor.matmul(
                        om[:, r, :], ones_bf[:, :P], be_bf,
                        start=False, stop=True,
                    )
                sl = osb[:, h * HT : (h + 1) * HT, :]
                if h == 0:
                    nc.scalar.copy(sl, om)
                else:
                    nc.vector.tensor_copy(sl, om)
            nc.sync.dma_start(
                out_et[rows, :].rearrange("(p r) d -> p r d", r=RT), osb
            )

    nc.finalize()
    return nc


_NC_CACHE = {}


def _get_nc(nch=NCH):
    if nch not in _NC_CACHE:
        _NC_CACHE[nch] = build(nch)
    return _NC_CACHE[nch]


def make_in_maps(inputs, n_cores=8):
    maps = []
    for b in range(n_cores):
        m = {
            "node_fts": np.ascontiguousarray(inputs["node_fts"][b]),
            "hidden": np.ascontiguousarray(inputs["hidden"][b]),
            "edge_fts": np.ascontiguousarray(inputs["edge_fts"][b]).reshape(NROW, F),
            "e_hidden": np.ascontiguousarray(inputs["e_hidden"][b]).reshape(NROW, F),
            "adj_mat": np.ascontiguousarray(inputs["adj_mat"][b]),
            "We": np.asarray(inputs["We"]),
            "be": np.asarray(inputs["be"]).reshape(1, D),
            "W_m1": np.asarray(inputs["W_m1"]),
            "b_m1": np.asarray(inputs["b_m1"]).reshape(1, D),
            "W_m2": np.asarray(inputs["W_m2"]),
            "b_m2": np.asarray(inputs["b_m2"]).reshape(1, D),
            "W_o1": np.asarray(inputs["W_o1"]),
            "b_o1": np.asarray(inputs["b_o1"]).reshape(1, D),
            "W_o2": np.asarray(inputs["W_o2"]),
            "b_o2": np.asarray(inputs["b_o2"]).reshape(1, D),
        }
        maps.append(m)
    return maps


def kernel(**inputs):
    from concourse.bass_utils import run_bass_kernel_spmd

    nc = _get_nc()
    in_maps = make_in_maps(inputs)
    res = run_bass_kernel_spmd(nc, in_maps, core_ids=list(range(B)))
    ret = np.stack([res.results[b]["out_ret"] for b in range(B)])
    et = np.stack(
        [res.results[b]["out_et"].reshape(N, N, D) for b in range(B)]
    )
    return ret, et


# revision 17
# speedup vs baseline: 1.6910x; 1.6910x over previous
"""Trainium2 Bass kernel for AlignedMPNN (gnn_message_passing).

Data-parallel over batch B=8 across 8 NeuronCores; each core computes one
batch element end-to-end (no collectives). Per core:

  et  = concat([edge_fts, e_hidden], -1) @ We + be          # [N*N, D]  (heavy)
  nt  = concat([node_fts, hidden], -1) (+ virtual zero row) # [N+1, 2F]
  msg1 = nt @ W_m1 + b_m1 ; msg2 = nt @ W_m2 + b_m2
  max2[j, d] = max_i (msg2[i, d] + (adjT[j, i] - 1) * 1e6)  # masked max
  ret = nt @ W_o1 + b_o1 + (msg1 + max2) @ W_o2 + b_o2      # rows 0..N-1

The et matmul contracts over the feature dim, so edge tiles are cast to
bf16 and transposed on the TensorEngine (1 cyc/row + fast weight load),
then multiplied by bf16 We with fp32 PSUM accumulation. The bias is
folded into the PSUM->SBUF drain as a tensor_tensor add against a
broadcast bias tile. Edge rows map to partitions contiguously
(row = chunk*CH + p*RT + r) so every DMA descriptor covers RT*512 bytes;
the same permutation is used on load and store so DRAM layout is exact.

Masked max runs on the VectorEngine as one fused scalar_tensor_tensor
per (i, j-tile); msg2 rows are broadcast across partitions by K=1
matmuls against a ones row (msg2 is flattened onto partition 0 via a
DRAM bounce so rows are base-partition-0 slices).
"""

import numpy as np
from contextlib import ExitStack

import concourse.bass as bass
from concourse import bacc
import concourse.tile as tile
from concourse import mybir
from concourse.masks import make_identity

B, N, F, D = 8, 256, 128, 128
P = 128
TF = 2 * F          # 256 = concat feature dim
NROW = N * N        # 65536 edge rows per batch element
CH = 1024           # edge rows per pipeline chunk
RT = CH // P        # 8 row-tiles per chunk
HT = RT // 2        # 4 row-tiles per half chunk (one PSUM bank)
NCH = NROW // CH    # 64 chunks
NODES = N + 1       # 257 (with virtual node)
BIGC = 1000000.0
NEG = -1.0e30
LSE_T = 40.0
LN2 = 0.6931471805599453
f32 = mybir.dt.float32
bf16 = mybir.dt.bfloat16
i32 = mybir.dt.int32
ADD = mybir.AluOpType.add
MAX = mybir.AluOpType.max
MULT = mybir.AluOpType.mult


def build(nch=NCH):
    nc = bacc.Bacc(None)

    node = nc.declare_dram_parameter("node_fts", [N, F], f32, isOutput=False)
    hid = nc.declare_dram_parameter("hidden", [N, F], f32, isOutput=False)
    edge = nc.declare_dram_parameter("edge_fts", [NROW, F], f32, isOutput=False)
    ehid = nc.declare_dram_parameter("e_hidden", [NROW, F], f32, isOutput=False)
    adj = nc.declare_dram_parameter("adj_mat", [N, N], i32, isOutput=False)
    We = nc.declare_dram_parameter("We", [TF, D], f32, isOutput=False)
    be = nc.declare_dram_parameter("be", [1, D], f32, isOutput=False)
    Wm1 = nc.declare_dram_parameter("W_m1", [TF, D], f32, isOutput=False)
    bm1 = nc.declare_dram_parameter("b_m1", [1, D], f32, isOutput=False)
    Wm2 = nc.declare_dram_parameter("W_m2", [TF, D], f32, isOutput=False)
    bm2 = nc.declare_dram_parameter("b_m2", [1, D], f32, isOutput=False)
    Wo1 = nc.declare_dram_parameter("W_o1", [TF, D], f32, isOutput=False)
    bo1 = nc.declare_dram_parameter("b_o1", [1, D], f32, isOutput=False)
    Wo2 = nc.declare_dram_parameter("W_o2", [D, D], f32, isOutput=False)
    bo2 = nc.declare_dram_parameter("b_o2", [1, D], f32, isOutput=False)
    out_et = nc.declare_dram_parameter("out_et", [NROW, D], f32, isOutput=True)
    out_ret = nc.declare_dram_parameter("out_ret", [N, D], f32, isOutput=True)

    with ExitStack() as ctx:
        tc = ctx.enter_context(tile.TileContext(nc))
        const = ctx.enter_context(tc.tile_pool(name="const", bufs=1))
        small = ctx.enter_context(tc.tile_pool(name="small", bufs=1))
        bfp = ctx.enter_context(tc.tile_pool(name="bfp", bufs=3))
        ctp = ctx.enter_context(tc.tile_pool(name="ctp", bufs=3))
        outp = ctx.enter_context(tc.tile_pool(name="outp", bufs=4))
        ps_t = ctx.enter_context(tc.tile_pool(name="ps_t", bufs=4, space="PSUM"))
        ps_mm = ctx.enter_context(tc.tile_pool(name="ps_mm", bufs=2, space="PSUM"))
        ps_sm = ctx.enter_context(tc.tile_pool(name="ps_sm", bufs=2, space="PSUM"))

        # ---- constants / weights ----
        ident = const.tile([P, P], f32)
        make_identity(nc, ident)
        ident_bf = const.tile([P, P], bf16)
        make_identity(nc, ident_bf)
        ones = const.tile([1, NODES], f32)
        nc.gpsimd.memset(ones, 1.0)

        def load_w2(dram_ap, name):
            t = const.tile([P, 2, D], f32, tag=name)
            nc.sync.dma_start(t, dram_ap.rearrange("(k p) d -> p k d", p=P))
            return t

        We_sb = load_w2(We, "We_sb")
        We_bf = const.tile([P, 2, D], bf16)
        nc.vector.tensor_copy(We_bf, We_sb)
        Wm1_sb = load_w2(Wm1, "Wm1_sb")
        Wm2_sb = load_w2(Wm2, "Wm2_sb")
        Wo1_sb = load_w2(Wo1, "Wo1_sb")
        Wo2_sb = const.tile([P, D], f32)
        nc.sync.dma_start(Wo2_sb, Wo2[:, :])

        def load_b(dram_ap, name):
            t = const.tile([1, D], f32, tag=name)
            nc.sync.dma_start(t, dram_ap[:, :])
            return t

        be_sb = load_b(be, "be_sb")
        bm1_sb = load_b(bm1, "bm1_sb")
        bm2_sb = load_b(bm2, "bm2_sb")
        bo1_sb = load_b(bo1, "bo1_sb")
        bo2_sb = load_b(bo2, "bo2_sb")

        bex = ps_sm.tile([P, D], f32, tag="sm")
        nc.tensor.matmul(bex, ones[:, :P], be_sb, start=True, stop=True)
        be_bcast = const.tile([P, D], f32)
        nc.scalar.copy(be_bcast, bex)

        # ---- small inputs ----
        nf_sb = small.tile([P, 2, F], f32)
        nc.sync.dma_start(nf_sb, node.rearrange("(t p) f -> p t f", p=P))
        hd_sb = small.tile([P, 2, F], f32)
        nc.sync.dma_start(hd_sb, hid.rearrange("(t p) f -> p t f", p=P))
        adj_i = small.tile([P, 2, N], i32)
        nc.sync.dma_start(adj_i, adj.rearrange("(t p) j -> p t j", p=P))
        adjf = small.tile([P, 2, N], f32)
        nc.vector.tensor_copy(adjf[:, 0, :], adj_i[:, 0, :])
        nc.vector.tensor_copy(adjf[:, 1, :], adj_i[:, 1, :])

        # ---- ntT[f, j]: transposed node features (f split in 2 halves) ----
        ntT = small.tile([P, 2, NODES], f32)
        nc.gpsimd.memset(ntT[:, 0, N : N + 1], 0.0)
        nc.gpsimd.memset(ntT[:, 1, N : N + 1], 0.0)
        for half, src in ((0, nf_sb), (1, hd_sb)):
            for jt in range(2):
                ps = ps_sm.tile([P, P], f32, tag="sm")
                nc.tensor.transpose(ps, src[:, jt, :], ident)
                nc.scalar.copy(ntT[:, half, jt * P : (jt + 1) * P], ps)

        # ---- msg1 (natural) and msg2T (transposed) ----
        msg1_sb = small.tile([P, 2, D], f32)
        for jt in range(2):
            ps = ps_sm.tile([P, D], f32, tag="sm")
            jsl = ntT[:, :, jt * P : (jt + 1) * P]
            nc.tensor.matmul(ps, jsl[:, 0, :], Wm1_sb[:, 0, :], start=True, stop=False)
            nc.tensor.matmul(ps, jsl[:, 1, :], Wm1_sb[:, 1, :], start=False, stop=False)
            nc.tensor.matmul(ps, ones[:, :P], bm1_sb, start=False, stop=True)
            nc.scalar.copy(msg1_sb[:, jt, :], ps)
        msg2T = small.tile([P, NODES], f32)
        psT = ps_sm.tile([P, NODES], f32, tag="sm")
        nc.tensor.matmul(psT, Wm2_sb[:, 0, :], ntT[:, 0, :], start=True, stop=False)
        nc.tensor.matmul(psT, Wm2_sb[:, 1, :], ntT[:, 1, :], start=False, stop=False)
        nc.tensor.matmul(psT, bm2_sb, ones[:, :NODES], start=False, stop=True)
        nc.scalar.copy(msg2T, psT)

        # ---- masked max over i via log-sum-exp (t=LSE_T, exact to ~log(2)/t
        # on ties; the virtual node guarantees a term >= exp(t*(bm2-c))) ----
        maxd = small.tile([P, 1], f32)
        nc.vector.tensor_reduce(maxd, msg2T, op=MAX, axis=mybir.AxisListType.X)
        cvec = small.tile([P, 1], f32)
        nc.vector.tensor_scalar(cvec, maxd, msg2T[:, N : N + 1], 0.5, ADD, MULT)
        ncv = small.tile([P, 1], f32)
        nc.vector.tensor_scalar(ncv, cvec, -LSE_T, None, MULT)
        ETs = small.tile([P, NODES], f32)
        nc.scalar.activation(
            ETs, msg2T, mybir.ActivationFunctionType.Exp, bias=ncv, scale=LSE_T
        )
        Esb = small.tile([P, 2, P], f32)
        for it in range(2):
            ps = ps_sm.tile([P, P], f32, tag="sm")
            nc.tensor.transpose(ps, ETs[:, it * P : (it + 1) * P], ident)
            nc.scalar.copy(Esb[:, it, :], ps)
        ps_ev = ps_sm.tile([1, P], f32, tag="sm")
        nc.tensor.transpose(ps_ev, ETs[:, N : N + 1], ident)
        E_virt = small.tile([1, P], f32)
        nc.scalar.copy(E_virt, ps_ev)
        ps_cr = ps_sm.tile([1, P], f32, tag="sm")
        nc.tensor.transpose(ps_cr, cvec, ident)
        c_row = small.tile([1, P], f32)
        nc.scalar.copy(c_row, ps_cr)
        ps_cb = ps_sm.tile([P, P], f32, tag="sm")
        nc.tensor.matmul(ps_cb, ones[:, :P], c_row, start=True, stop=True)
        c_adj = small.tile([P, P], f32)
        # c - 127*ln2/t : folds the f32 exponent bias into the ln recombine
        nc.vector.tensor_scalar(c_adj, ps_cb, -127.0 * LN2 / LSE_T, None, ADD)

        acc = small.tile([P, 2, D], f32)
        for jt in range(2):
            jsl = slice(jt * P, (jt + 1) * P)
            ps = ps_sm.tile([P, D], f32, tag="sm")
            nc.tensor.matmul(ps, adjf[:, 0, jsl], Esb[:, 0, :], start=True, stop=False)
            nc.tensor.matmul(ps, adjf[:, 1, jsl], Esb[:, 1, :], start=False, stop=False)
            nc.tensor.matmul(ps, ones[:, :P], E_virt, start=False, stop=True)
            # ln(S) = ln(mantissa) + exp_field*ln2 - 127*ln2  (S > 0, normal)
            si = ps.bitcast(i32)
            eb = small.tile([P, D], i32, tag="eb")
            nc.vector.tensor_scalar(
                eb, si, 23, None, mybir.AluOpType.arith_shift_right
            )
            ebf = small.tile([P, D], f32, tag="ebf")
            nc.vector.tensor_copy(ebf, eb)
            mi = small.tile([P, D], i32, tag="mi")
            nc.vector.tensor_scalar(
                mi, si, 0x007FFFFF, 0x3F800000,
                mybir.AluOpType.bitwise_and, mybir.AluOpType.bitwise_or,
            )
            lnm = small.tile([P, D], f32, tag="lnm")
            nc.scalar.activation(
                lnm, mi.bitcast(f32), mybir.ActivationFunctionType.Ln
            )
            tmp = small.tile([P, D], f32, tag="tmpl")
            nc.vector.scalar_tensor_tensor(tmp, ebf, LN2 / LSE_T, c_adj, MULT, ADD)
            nc.vector.scalar_tensor_tensor(
                acc[:, jt, :], lnm, 1.0 / LSE_T, tmp, MULT, ADD
            )

        # ---- msgs = msg1 + acc; ret = nt@Wo1 + msgs@Wo2 + bo1 + bo2 ----
        msgs = small.tile([P, 2, D], f32)
        msgsT = small.tile([P, 2, P], f32)
        for jt in range(2):
            nc.vector.tensor_add(msgs[:, jt, :], msg1_sb[:, jt, :], acc[:, jt, :])
            ps = ps_sm.tile([P, P], f32, tag="sm")
            nc.tensor.transpose(ps, msgs[:, jt, :], ident)
            nc.scalar.copy(msgsT[:, jt, :], ps)
        for jt in range(2):
            ps = ps_sm.tile([P, D], f32, tag="sm")
            jsl = ntT[:, :, jt * P : (jt + 1) * P]
            nc.tensor.matmul(ps, jsl[:, 0, :], Wo1_sb[:, 0, :], start=True, stop=False)
            nc.tensor.matmul(ps, jsl[:, 1, :], Wo1_sb[:, 1, :], start=False, stop=False)
            nc.tensor.matmul(ps, msgsT[:, jt, :], Wo2_sb, start=False, stop=False)
            nc.tensor.matmul(ps, ones[:, :P], bo1_sb, start=False, stop=False)
            nc.tensor.matmul(ps, ones[:, :P], bo2_sb, start=False, stop=True)
            rsb = small.tile([P, D], f32, tag="rsb")
            nc.scalar.copy(rsb, ps)
            nc.sync.dma_start(out_ret[jt * P : (jt + 1) * P, :], rsb)

        be_b4 = be_bcast[:, None, :].to_broadcast([P, HT, D])
        # ---- et stream: casting DMA (f32->bf16) -> PE transpose -> matmul
        #      (+ K=1 bias matmul) -> drain -> store.
        #      Row permutation: row = c*CH + p*RT + r
        for c in range(nch):
            rows = slice(c * CH, (c + 1) * CH)
            ebf = bfp.tile([P, RT, F], bf16, tag="ebf")
            nc.gpsimd.dma_start(ebf, edge[rows, :].rearrange("(p r) f -> p r f", r=RT))
            hbf = bfp.tile([P, RT, F], bf16, tag="hbf")
            nc.gpsimd.dma_start(hbf, ehid[rows, :].rearrange("(p r) f -> p r f", r=RT))

            cte = ctp.tile([P, RT, P], bf16, tag="cte")
            cth = ctp.tile([P, RT, P], bf16, tag="cth")
            for src, dst, eng in ((ebf, cte, 0), (hbf, cth, 1)):
                for h in range(2):
                    ps = ps_t.tile([P, HT, P], bf16, tag="pst")
                    for r in range(HT):
                        nc.tensor.transpose(ps[:, r, :], src[:, h * HT + r, :], ident_bf)
                    sl = dst[:, h * HT : (h + 1) * HT, :]
                    if eng == 0 and h == 0:
                        nc.vector.tensor_copy(sl, ps)
                    else:
                        nc.scalar.copy(sl, ps)

            osb = outp.tile([P, RT, D], f32, tag="osb")
            for h in range(2):
                om = ps_mm.tile([P, HT, D], f32, tag="om")
                for r in range(HT):
                    rr = h * HT + r
                    nc.tensor.matmul(
                        om[:, r, :], cte[:, rr, :], We_bf[:, 0, :],
                        start=True, stop=False,
                    )
                    nc.tensor.matmul(
                        om[:, r, :], cth[:, rr, :], We_bf[:, 1, :],
                        start=False, stop=True,
                    )
                sl = osb[:, h * HT : (h + 1) * HT, :]
                nc.vector.tensor_tensor(sl, om, be_b4, ADD)
            nc.sync.dma_start(
                out_et[rows, :].rearrange("(p r) d -> p r d", r=RT), osb
            )

    nc.finalize()
    return nc


_NC_CACHE = {}


def _get_nc(nch=NCH):
    if nch not in _NC_CACHE:
        _NC_CACHE[nch] = build(nch)
    return _NC_CACHE[nch]


def make_in_maps(inputs, n_cores=8):
    maps = []
    for b in range(n_cores):
        m = {
            "node_fts": np.ascontiguousarray(inputs["node_fts"][b]),
            "hidden": np.ascontiguousarray(inputs["hidden"][b]),
            "edge_fts": np.ascontiguousarray(inputs["edge_fts"][b]).reshape(NROW, F),
            "e_hidden": np.ascontiguousarray(inputs["e_hidden"][b]).reshape(NROW, F),
            "adj_mat": np.ascontiguousarray(inputs["adj_mat"][b]),
            "We": np.asarray(inputs["We"]),
            "be": np.asarray(inputs["be"]).reshape(1, D),
            "W_m1": np.asarray(inputs["W_m1"]),
            "b_m1": np.asarray(inputs["b_m1"]).reshape(1, D),
            "W_m2": np.asarray(inputs["W_m2"]),
            "b_m2": np.asarray(inputs["b_m2"]).reshape(1, D),
            "W_o1": np.asarray(inputs["W_o1"]),
            "b_o1": np.asarray(inputs["b_o1"]).reshape(1, D),
            "W_o2": np.asarray(inputs["W_o2"]),
            "b_o2": np.asarray(inputs["b_o2"]).reshape(1, D),
        }
        maps.append(m)
    return maps


def kernel(**inputs):
    from concourse.bass_utils import run_bass_kernel_spmd

    nc = _get_nc()
    in_maps = make_in_maps(inputs)
    res = run_bass_kernel_spmd(nc, in_maps, core_ids=list(range(B)))
    ret = np.stack([res.results[b]["out_ret"] for b in range(B)])
    et = np.stack(
        [res.results[b]["out_et"].reshape(N, N, D) for b in range(B)]
    )
    return ret, et
